# revision 1
# baseline (speedup 1.0000x reference)
"""Trainium2 Bass kernel for nn_BertCounterFactTransformer.

Contract: kernel(**inputs) takes FULL unsharded numpy inputs (as produced by
reference.setup_inputs()) and returns the FULL [32, 1024] float32 output.

Strategy (data-parallel over batch, 8 cores x 4 samples):
  - Host: compute false/option masks + per-sample-slot tile bounds from x_ids,
    transpose x to xT, shard over cores. The program is specialized to the
    bounds (max over cores per slot -> one SPMD program) and cached per
    bounds tuple; masks keep any extra computed tiles harmless.
  - Device, per sample (F = #false row tiles, OJ = first option col):
      qT projections      only cols [0, 128F)        (bf16 matmuls)
      kT projections      only cols [OJ, 512)
      gate                g = exp(al)*fmask / max(sum, 1e-8)
      scores              [128F, 512-OJ] block only   (3 types)
      E_sup = exp(S_sup/32 + obias), E_rep = exp(S_rep/32 + tanh(S_con/32) + obias)
      coeff_t = gate / rowsum(E_t);  r_t = coeff_t^T @ E_t
      pooled  = x^T @ [gate, r_rep, r_sup]  -> fused^T columns   (f32 matmuls)
  - Device, batched tail in f32: h=relu(W1^T fused + b1), y=W2^T h + b2, LN.

Key identity: gate @ (attn @ x) == (gate @ attn) @ x, so [L,D] attention
outputs are never materialized. Column masking is injected into PSUM via K=1
matmul bias rows (obias = -960 raw -> -30 after the 1/32 scale).
"""

import sys

if "/opt/trn_rl_repo" not in sys.path:
    sys.path.insert(0, "/opt/trn_rl_repo")

import numpy as np
import ml_dtypes
from contextlib import ExitStack

np_bf16 = ml_dtypes.bfloat16

import concourse.bacc as bacc
import concourse.bass as bass
import concourse.mybir as mybir
import concourse.tile as tile
from concourse import bass_utils

f32 = mybir.dt.float32
bf16 = mybir.dt.bfloat16
AF = mybir.ActivationFunctionType
ALU = mybir.AluOpType

B, L, D = 32, 512, 1024
NCORES = 8
BC = B // NCORES          # samples per core
NL = L // 128             # 4 L-tiles
ND = D // 128             # 8 D-tiles
NC3 = 3 * D // 128        # 24 tiles of the 3D fused dim
SCALE = 1.0 / 32.0        # 1/sqrt(D)
OBIAS_RAW = -960.0        # -30 after * SCALE
LN_EPS = 1e-5

PROJ_NAMES = ["w_sq", "w_sk", "w_cq", "w_ck", "w_rq", "w_rk"]
PBIAS_NAMES = ["b_sq", "b_sk", "b_cq", "b_ck", "b_rq", "b_rk"]
QS, KS, QC, KC, QR, KR = range(6)
QPROJ = (QS, QC, QR)

_PROGRAM_CACHE = {}
_M_CACHE = {}


def _m_matrix(wq, wk):
    import hashlib
    wq = np.asarray(wq, dtype=np.float32)
    wk = np.asarray(wk, dtype=np.float32)
    key = hashlib.blake2b(wq.tobytes() + wk.tobytes(), digest_size=16).digest()
    if key not in _M_CACHE:
        _M_CACHE[key] = np.ascontiguousarray(wq @ wk.T).astype(np_bf16)
    return _M_CACHE[key]


def build_program(bounds=((2, 2),) * BC, use_m=True, enable_asserts=False):
    """bounds[s] = (F, J0): false rows live in tiles [0,F), option cols in
    [128*J0, 512). Computing a superset is always correct (masks zero it)."""
    nc = bacc.Bacc(
        "TRN2",
        target_bir_lowering=False,
        debug=False,
        enable_asserts=enable_asserts,
        num_devices=NCORES,
    )

    xT_d = nc.dram_tensor("xT", [BC, D, L], bf16, kind="ExternalInput").ap()
    x_d = nc.dram_tensor("x", [BC, L, D], f32, kind="ExternalInput").ap()
    fmask_d = nc.dram_tensor("fmask", [BC, L], f32, kind="ExternalInput").ap()
    obias_d = nc.dram_tensor("obias", [BC, L], bf16, kind="ExternalInput").ap()

    if use_m:
        W_d = {p: nc.dram_tensor(n, [D, D], bf16, kind="ExternalInput").ap()
               for p, n in ((QS, "m_sup"), (QC, "m_con"), (QR, "m_rep"))}
    else:
        W_d = {p: nc.dram_tensor(PROJ_NAMES[p], [D, D], bf16, kind="ExternalInput").ap()
               for p in range(6)}
    Brow_d = {} if use_m else {
        p: nc.dram_tensor(PBIAS_NAMES[p], [1, D], bf16, kind="ExternalInput").ap()
        for p in range(6)}
    wanom_d = nc.dram_tensor("w_anom", [D, 1], bf16, kind="ExternalInput").ap()
    wf1_d = nc.dram_tensor("w_f1", [ND, 128, NC3 * 128], bf16, kind="ExternalInput").ap()
    wf2_d = nc.dram_tensor("w_f2", [ND, 128, ND * 128], bf16, kind="ExternalInput").ap()
    bf1_d = nc.dram_tensor("b_f1", [128, ND], f32, kind="ExternalInput").ap()
    bf2_d = nc.dram_tensor("b_f2", [128, ND], f32, kind="ExternalInput").ap()
    lng_d = nc.dram_tensor("ln_g", [128, ND], f32, kind="ExternalInput").ap()
    lnb_d = nc.dram_tensor("ln_b", [128, ND], f32, kind="ExternalInput").ap()

    out_d = nc.dram_tensor("out", [BC, D], f32, kind="ExternalOutput").ap()

    with tile.TileContext(nc) as tc, ExitStack() as ctx:
        const_p = ctx.enter_context(tc.tile_pool(name="const", bufs=1))
        tmp_p = ctx.enter_context(tc.tile_pool(name="tmp", bufs=2))
        sm_p = ctx.enter_context(tc.tile_pool(name="small", bufs=3))
        tail_p = ctx.enter_context(tc.tile_pool(name="tail", bufs=1))
        ps_big = ctx.enter_context(tc.tile_pool(name="psb", bufs=4, space="PSUM"))
        ps_s = ctx.enter_context(tc.tile_pool(name="pss", bufs=4, space="PSUM"))
        es2 = ExitStack()   # closed after phase C: x, E
        x_p = es2.enter_context(tc.tile_pool(name="x", bufs=3))
        e_p = es2.enter_context(tc.tile_pool(name="emat", bufs=2))
        es1 = ExitStack()   # closed after phase B: xT, W, proj
        xT_p = es1.enter_context(tc.tile_pool(name="xT", bufs=1))
        w_p = es1.enter_context(tc.tile_pool(name="w", bufs=2))
        proj_p = es1.enter_context(tc.tile_pool(name="proj", bufs=1))

        # ---- constants ----
        ones_row = const_p.tile([1, L], bf16)
        nc.vector.memset(ones_row[:], 1.0)
        ones_f = const_p.tile([1, 128], f32)
        nc.vector.memset(ones_f[:], 1.0)
        ones_col = const_p.tile([128, 1], f32)
        nc.vector.memset(ones_col[:], 1.0)
        iot_t = const_p.tile([128, 128], mybir.dt.int32)
        nc.gpsimd.iota(iot_t[:], pattern=[[1, 128]], base=0, channel_multiplier=-1)
        ident_t = const_p.tile([128, 128], f32)
        nc.vector.tensor_scalar(ident_t[:], iot_t[:], scalar1=0, scalar2=None,
                                op0=ALU.is_equal)

        wanom_t = const_p.tile([128, ND], bf16)
        nc.scalar.dma_start(wanom_t[:], wanom_d[:, 0].rearrange("(k p) -> p k", p=128))
        brow_t = {}
        for p in Brow_d:
            brow_t[p] = const_p.tile([1, D], bf16, name=f"brow{p}")
            nc.sync.dma_start(brow_t[p][:], Brow_d[p][:])
        bf1_t = const_p.tile([128, ND], f32)
        nc.scalar.dma_start(bf1_t[:], bf1_d[:])
        bf2_t = const_p.tile([128, ND], f32)
        nc.scalar.dma_start(bf2_t[:], bf2_d[:])
        lng_t = const_p.tile([128, ND], f32)
        nc.scalar.dma_start(lng_t[:], lng_d[:])
        lnb_t = const_p.tile([128, ND], f32)
        nc.scalar.dma_start(lnb_t[:], lnb_d[:])

        fusedT = tail_p.tile([128, NC3, BC], bf16)

        # per-slot geometry
        geo = []
        for s in range(BC):
            F, J0 = bounds[s]
            geo.append((F, J0, F * 128, J0 * 128, L - J0 * 128,
                        F > 0 and L - J0 * 128 > 0))

        # ---- Phase A: xT resident + gates; M weights via one DMA each ----
        xT_t = xT_p.tile([128, BC * ND, L], bf16)
        fm_ts, ob_ts, x_ts = [], [], []
        for s in range(BC):
            nc.sync.dma_start(
                xT_t[:, s * ND : (s + 1) * ND, :],
                xT_d[s].rearrange("(k p) i -> p k i", p=128),
            )
            fm_t = sm_p.tile([128, NL], f32, tag="fm", bufs=BC, name=f"fm{s}")
            nc.scalar.dma_start(fm_t[:], fmask_d[s].rearrange("(t p) -> p t", p=128))
            fm_ts.append(fm_t)
            ob_t = sm_p.tile([1, L], bf16, tag="ob", bufs=2, name=f"ob{s}")
            nc.scalar.dma_start(ob_t[:], obias_d[s : s + 1, :])
            ob_ts.append(ob_t)

        gate_ts = []
        for s in range(BC):
            F, J0, CQ, OJ, NO, have_attn = geo[s]
            gate_t = sm_p.tile([128, NL], f32, tag="gate", bufs=BC, name=f"gate{s}")
            gate_ts.append(gate_t)
            if F == 0:
                continue
            ghat_t = sm_p.tile([128, NL], f32, tag="ghat")
            for it in range(F):
                al_ps = ps_s.tile([128, 1], f32, tag="pss")
                for k in range(ND):
                    nc.tensor.matmul(
                        al_ps[:],
                        lhsT=xT_t[:, s * ND + k, it * 128 : (it + 1) * 128],
                        rhs=wanom_t[:, k : k + 1],
                        start=(k == 0), stop=(k == ND - 1),
                    )
                eg_t = sm_p.tile([128, 1], f32, tag="eg")
                nc.scalar.activation(eg_t[:], al_ps[:], AF.Exp)
                nc.vector.tensor_mul(
                    ghat_t[:, it : it + 1], eg_t[:], fm_ts[s][:, it : it + 1]
                )
            gsum_t = sm_p.tile([128, 1], f32, tag="gsum")
            nc.vector.tensor_reduce(
                gsum_t[:], ghat_t[:, 0:F], axis=mybir.AxisListType.X, op=ALU.add
            )
            S_ps = ps_s.tile([1, 1], f32, tag="pss")
            nc.tensor.matmul(S_ps[:], lhsT=gsum_t[:], rhs=ones_col[:],
                             start=True, stop=True)
            Smax_t = sm_p.tile([1, 1], f32, tag="Smax")
            nc.vector.tensor_scalar_max(Smax_t[:], S_ps[:], 1e-8)
            Sb_ps = ps_s.tile([128, 1], f32, tag="pss")
            nc.tensor.matmul(Sb_ps[:], lhsT=ones_f[:], rhs=Smax_t[:],
                             start=True, stop=True)
            recipS_t = sm_p.tile([128, 1], f32, tag="recipS")
            nc.vector.reciprocal(recipS_t[:], Sb_ps[:])
            nc.vector.tensor_scalar_mul(gate_t[:, 0:F], ghat_t[:, 0:F],
                                        recipS_t[:])

        # ---- projections: one gpsimd DMA per M matrix, all samples inner ----
        projs = [[None] * BC for _ in range(6)]
        proj_list = list(QPROJ) if use_m else list(range(6))
        for p in proj_list:
            qside = p in QPROJ
            widths = [
                ((g[2] if qside else g[4]) if g[5] else 0) for g in geo
            ]
            wmax = max(widths)
            if wmax == 0:
                continue
            wt = w_p.tile([128, ND, D], bf16, tag="w", name=f"w{p}")
            nc.gpsimd.dma_start(wt[:], W_d[p].rearrange("(k p) c -> p k c", p=128))
            pt = proj_p.tile([128, BC, ND, wmax], bf16, tag=f"proj{p}")
            for m in range(ND):
                for s in range(BC):
                    width = widths[s]
                    if width == 0:
                        continue
                    lo = 0 if qside else geo[s][3]
                    ps = ps_big.tile([128, width], f32, tag="ps")
                    for k in range(ND):
                        nc.tensor.matmul(
                            ps[:], lhsT=wt[:, k, m * 128 : (m + 1) * 128],
                            rhs=xT_t[:, s * ND + k, lo : lo + width],
                            start=(k == 0), stop=(use_m and k == ND - 1),
                        )
                    if not use_m:
                        nc.tensor.matmul(
                            ps[:], lhsT=brow_t[p][:, m * 128 : (m + 1) * 128],
                            rhs=ones_row[:, 0:width], start=False, stop=True,
                        )
                    nc.vector.tensor_copy(pt[:, s, m, :], ps[:])
            for s in range(BC):
                if widths[s]:
                    projs[p][s] = pt

        for s in range(BC):
            x_t = x_p.tile([128, NL, D], f32, tag="x", name=f"x{s}")
            nc.sync.dma_start(x_t[:], x_d[s].rearrange("(t p) d -> p t d", p=128))
            x_ts.append(x_t)

        # ---- Phase B: scores -> E, coeffs (all samples) ----
        E_sups, E_reps, co_sups, co_reps = {}, {}, {}, {}
        for s in range(BC):
            F, J0, CQ, OJ, NO, have_attn = geo[s]
            if not have_attn:
                continue
            E_sup = e_p.tile([128, max(F, 1), NO], f32, tag="esup", bufs=BC,
                             name=f"esup{s}")
            E_rep = e_p.tile([128, max(F, 1), NO], f32, tag="erep", bufs=BC,
                             name=f"erep{s}")
            co_sup = sm_p.tile([128, NL], f32, tag="cosup", bufs=BC,
                               name=f"cosup{s}")
            co_rep = sm_p.tile([128, NL], f32, tag="corep", bufs=BC,
                               name=f"corep{s}")
            E_sups[s], E_reps[s] = E_sup, E_rep
            co_sups[s], co_reps[s] = co_sup, co_rep
            gate_t = gate_ts[s]
            ob_t = ob_ts[s]
            for it in range(F):
                isl = slice(it * 128, (it + 1) * 128)
                ps_sup = ps_big.tile([128, NO], f32, tag="ps")
                for k in range(ND):
                    nc.tensor.matmul(
                        ps_sup[:], lhsT=projs[QS][s][:, s, k, isl],
                        rhs=(xT_t[:, s * ND + k, OJ:L] if use_m
                             else projs[KS][s][:, s, k, 0:NO]),
                        start=(k == 0), stop=False,
                    )
                nc.tensor.matmul(ps_sup[:], lhsT=ones_row[:, 0:128],
                                 rhs=ob_t[:, OJ:L], start=False, stop=True)
                ps_con = ps_big.tile([128, NO], f32, tag="ps")
                for k in range(ND):
                    nc.tensor.matmul(
                        ps_con[:], lhsT=projs[QC][s][:, s, k, isl],
                        rhs=(xT_t[:, s * ND + k, OJ:L] if use_m
                             else projs[KC][s][:, s, k, 0:NO]),
                        start=(k == 0), stop=(k == ND - 1),
                    )
                ps_rep = ps_big.tile([128, NO], f32, tag="ps")
                for k in range(ND):
                    nc.tensor.matmul(
                        ps_rep[:], lhsT=projs[QR][s][:, s, k, isl],
                        rhs=(xT_t[:, s * ND + k, OJ:L] if use_m
                             else projs[KR][s][:, s, k, 0:NO]),
                        start=(k == 0), stop=False,
                    )
                nc.tensor.matmul(ps_rep[:], lhsT=ones_row[:, 0:128],
                                 rhs=ob_t[:, OJ:L], start=False, stop=True)

                T_t = tmp_p.tile([128, NO], f32, tag="T")
                nc.scalar.activation(T_t[:], ps_con[:], AF.Tanh, scale=SCALE)
                A_t = tmp_p.tile([128, NO], f32, tag="A")
                nc.vector.scalar_tensor_tensor(
                    A_t[:], in0=ps_rep[:], scalar=SCALE, in1=T_t[:],
                    op0=ALU.mult, op1=ALU.add,
                )
                rs_sup = sm_p.tile([128, 1], f32, tag="rssup")
                nc.scalar.activation(E_sup[:, it, :], ps_sup[:], AF.Exp,
                                     scale=SCALE, accum_out=rs_sup[:])
                rs_rep = sm_p.tile([128, 1], f32, tag="rsrep")
                nc.scalar.activation(E_rep[:, it, :], A_t[:], AF.Exp,
                                     accum_out=rs_rep[:])
                rc_sup = sm_p.tile([128, 1], f32, tag="rcsup")
                nc.vector.reciprocal(rc_sup[:], rs_sup[:])
                nc.vector.tensor_mul(co_sup[:, it : it + 1],
                                     gate_t[:, it : it + 1], rc_sup[:])
                rc_rep = sm_p.tile([128, 1], f32, tag="rcrep")
                nc.vector.reciprocal(rc_rep[:], rs_rep[:])
                nc.vector.tensor_mul(co_rep[:, it : it + 1],
                                     gate_t[:, it : it + 1], rc_rep[:])

        es1.close()

        # ---- Phase C: r vectors, G, pooled (all samples) ----
        for s in range(BC):
            F, J0, CQ, OJ, NO, have_attn = geo[s]
            x_t = x_ts[s]

            G_t = sm_p.tile([128, NL, 3], f32, tag="G")
            nc.vector.memset(G_t[:], 0.0)
            if F > 0:
                for it in range(F):
                    nc.vector.tensor_copy(G_t[:, it, 0:1],
                                          gate_ts[s][:, it : it + 1])
            if have_attn:
                E_sup, E_rep = E_sups[s], E_reps[s]
                co_sup, co_rep = co_sups[s], co_reps[s]
                for jt in range(J0, NL):
                    jsl = slice(jt * 128 - OJ, jt * 128 - OJ + 128)
                    r_ps = ps_s.tile([128, 2], f32, tag="pss")
                    for it in range(F):
                        nc.tensor.matmul(
                            r_ps[:, 0:1], lhsT=E_rep[:, it, jsl],
                            rhs=co_rep[:, it : it + 1],
                            start=(it == 0), stop=(it == F - 1),
                        )
                    for it in range(F):
                        nc.tensor.matmul(
                            r_ps[:, 1:2], lhsT=E_sup[:, it, jsl],
                            rhs=co_sup[:, it : it + 1],
                            start=(it == 0), stop=(it == F - 1),
                        )
                    nc.vector.tensor_copy(G_t[:, jt, 1:3], r_ps[:, 0:2])

            rts = sorted(set(range(F)) | (set(range(J0, NL)) if have_attn else set()))
            if not rts:
                rts = [0]
            for m in range(ND):
                pool_ps = ps_s.tile([128, 3], f32, tag="pss")
                for i, rt in enumerate(rts):
                    nc.tensor.matmul(
                        pool_ps[:], lhsT=x_t[:, rt, m * 128 : (m + 1) * 128],
                        rhs=G_t[:, rt, :],
                        start=(i == 0), stop=(i == len(rts) - 1),
                    )
                for t in range(3):
                    nc.vector.tensor_copy(
                        fusedT[:, t * ND + m, s : s + 1], pool_ps[:, t : t + 1]
                    )

        es2.close()

        # ---- batched MLP tail ----
        wf1_p = ctx.enter_context(tc.tile_pool(name="wf1", bufs=8))
        hT_t = tail_p.tile([128, ND, BC], bf16)
        for m in range(ND):
            wt = wf1_p.tile([128, NC3, 128], bf16, tag="wf1")
            nc.gpsimd.dma_start(wt[:], wf1_d[m].rearrange("p (k c) -> p k c", c=128))
            h_ps = ps_s.tile([128, BC], f32, tag="pss")
            for k in range(NC3):
                nc.tensor.matmul(h_ps[:], lhsT=wt[:, k, :], rhs=fusedT[:, k, :],
                                 start=(k == 0), stop=(k == NC3 - 1))
            nc.scalar.activation(hT_t[:, m, :], h_ps[:], AF.Relu,
                                 bias=bf1_t[:, m : m + 1])

        yT_t = tail_p.tile([128, ND, BC], f32)
        sq_t = tail_p.tile([128, ND, BC], f32)
        for m in range(ND):
            wt = wf1_p.tile([128, ND, 128], bf16, tag="wf2")
            nc.gpsimd.dma_start(wt[:], wf2_d[m].rearrange("p (k c) -> p k c", c=128))
            y_ps = ps_s.tile([128, BC], f32, tag="pss")
            for k in range(ND):
                nc.tensor.matmul(y_ps[:], lhsT=wt[:, k, :], rhs=hT_t[:, k, :],
                                 start=(k == 0), stop=(k == ND - 1))
            nc.vector.tensor_scalar_add(yT_t[:, m, :], y_ps[:], bf2_t[:, m : m + 1])
            nc.scalar.square(sq_t[:, m, :], yT_t[:, m, :])

        sum_ps = ps_s.tile([1, BC], f32, tag="pss")
        for m in range(ND):
            nc.tensor.matmul(sum_ps[:], lhsT=ones_col[:], rhs=yT_t[:, m, :],
                             start=(m == 0), stop=(m == ND - 1))
        ssq_ps = ps_s.tile([1, BC], f32, tag="pss")
        for m in range(ND):
            nc.tensor.matmul(ssq_ps[:], lhsT=ones_col[:], rhs=sq_t[:, m, :],
                             start=(m == 0), stop=(m == ND - 1))
        mean_t = sm_p.tile([1, BC], f32, tag="mean")
        nc.scalar.mul(mean_t[:], sum_ps[:], 1.0 / D)
        msq_t = sm_p.tile([1, BC], f32, tag="msq")
        nc.scalar.mul(msq_t[:], ssq_ps[:], 1.0 / D)
        m2_t = sm_p.tile([1, BC], f32, tag="m2")
        nc.vector.tensor_mul(m2_t[:], mean_t[:], mean_t[:])
        var_t = sm_p.tile([1, BC], f32, tag="var")
        nc.vector.tensor_sub(var_t[:], msq_t[:], m2_t[:])
        nc.vector.tensor_scalar_add(var_t[:], var_t[:], LN_EPS)
        sd_t = sm_p.tile([1, BC], f32, tag="sd")
        nc.scalar.sqrt(sd_t[:], var_t[:])
        rstd_t = sm_p.tile([1, BC], f32, tag="rstd")
        nc.vector.reciprocal(rstd_t[:], sd_t[:])

        mb_ps = ps_s.tile([128, BC], f32, tag="pss")
        nc.tensor.matmul(mb_ps[:], lhsT=ones_f[:], rhs=mean_t[:],
                         start=True, stop=True)
        mb_t = sm_p.tile([128, BC], f32, tag="mbt")
        nc.vector.tensor_copy(mb_t[:], mb_ps[:])
        rb_ps = ps_s.tile([128, BC], f32, tag="pss")
        nc.tensor.matmul(rb_ps[:], lhsT=ones_f[:], rhs=rstd_t[:],
                         start=True, stop=True)
        rb_t = sm_p.tile([128, BC], f32, tag="rbt")
        nc.vector.tensor_copy(rb_t[:], rb_ps[:])

        zrow_t = tail_p.tile([BC, D], f32)
        for m in range(ND):
            z_t = tmp_p.tile([128, BC], f32, tag="z")
            nc.vector.tensor_sub(z_t[:], yT_t[:, m, :], mb_t[:])
            nc.vector.tensor_mul(z_t[:], z_t[:], rb_t[:])
            z2_t = tmp_p.tile([128, BC], f32, tag="z2")
            nc.vector.tensor_scalar(
                z2_t[:], z_t[:], scalar1=lng_t[:, m : m + 1],
                scalar2=lnb_t[:, m : m + 1], op0=ALU.mult, op1=ALU.add,
            )
            tr_ps = ps_s.tile([BC, 128], f32, tag="pss")
            nc.tensor.transpose(tr_ps[:], z2_t[:], ident_t[:])
            nc.vector.tensor_copy(zrow_t[:, m * 128 : (m + 1) * 128], tr_ps[:])
        nc.sync.dma_start(out_d[:, :], zrow_t[:, :])

    nc.compile()
    return nc


def _host_prep(inputs):
    """Returns (in_maps, bounds)."""
    x = np.asarray(inputs["x"], dtype=np.float32)
    x_ids = np.asarray(inputs["x_ids"])
    pad_idx = int(np.asarray(inputs["pad_idx"]))
    sep_idx = int(np.asarray(inputs["sep_idx"]))
    assert x.shape == (B, L, D), x.shape

    valid = x_ids != pad_idx
    sepm = x_ids == sep_idx
    has = sepm.any(axis=1)
    first = sepm.argmax(axis=1)
    vlen = valid.sum(axis=1)
    fb = np.clip(vlen // 2, 1, max(1, L - 2))
    sp = np.where(has, first, fb)
    pos = np.arange(L)
    fmask = ((pos[None, :] < sp[:, None]) & valid).astype(np.float32)
    omask = (pos[None, :] > sp[:, None]) & valid
    obias = np.where(omask, 0.0, OBIAS_RAW).astype(np.float32)

    # per-slot tile bounds: F covers all false rows, J0 covers all option cols
    F_all = np.ceil(sp / 128).astype(int)           # false subset of [0, sep)
    J0_all = np.minimum((sp + 1) // 128, NL)        # option subset of [sep+1, L)
    bounds = tuple(
        (int(F_all.reshape(NCORES, BC)[:, s].max()),
         int(J0_all.reshape(NCORES, BC)[:, s].min()))
        for s in range(BC)
    )

    xT = np.ascontiguousarray(x.transpose(0, 2, 1))

    def w(name):
        return np.ascontiguousarray(np.asarray(inputs[name], dtype=np.float32))

    def ppart(name):
        return np.ascontiguousarray(np.asarray(inputs[name], dtype=np.float32)
                                    .reshape(ND, 128).T)

    use_m = all(not np.any(np.asarray(inputs[n])) for n in PBIAS_NAMES)
    shared = {}
    if use_m:
        for dst, qn, kn in (("m_sup", "w_sq", "w_sk"), ("m_con", "w_cq", "w_ck"),
                            ("m_rep", "w_rq", "w_rk")):
            shared[dst] = _m_matrix(inputs[qn], inputs[kn])
    else:
        for p in range(6):
            shared[PROJ_NAMES[p]] = w(PROJ_NAMES[p]).astype(np_bf16)
            shared[PBIAS_NAMES[p]] = w(PBIAS_NAMES[p]).reshape(1, D).astype(np_bf16)
    shared["w_anom"] = w("w_anom").reshape(D, 1).astype(np_bf16)
    def mpack(name, nk):
        a = w(name)                      # [nk*128, ND*128]
        a = a.reshape(nk, 128, ND, 128).transpose(2, 1, 0, 3).reshape(ND, 128, nk * 128)
        return np.ascontiguousarray(a).astype(np_bf16)

    shared["w_f1"] = mpack("w_f1", NC3)
    shared["w_f2"] = mpack("w_f2", ND)
    shared["b_f1"] = ppart("b_f1")
    shared["b_f2"] = ppart("b_f2")
    shared["ln_g"] = ppart("ln_g")
    shared["ln_b"] = ppart("ln_b")

    in_maps = []
    for c in range(NCORES):
        sl = slice(c * BC, (c + 1) * BC)
        m = dict(shared)
        m["x"] = np.ascontiguousarray(x[sl])
        m["xT"] = np.ascontiguousarray(xT[sl]).astype(np_bf16)
        m["fmask"] = np.ascontiguousarray(fmask[sl])
        m["obias"] = np.ascontiguousarray(obias[sl]).astype(np_bf16)
        in_maps.append(m)
    return in_maps, bounds, use_m


def get_program(bounds, use_m):
    key = (bounds, use_m)
    if key not in _PROGRAM_CACHE:
        _PROGRAM_CACHE[key] = build_program(bounds, use_m=use_m)
    return _PROGRAM_CACHE[key]


def run(trace=False, **inputs):
    in_maps, bounds, use_m = _host_prep(inputs)
    nc = get_program(bounds, use_m)
    res = bass_utils.run_bass_kernel_spmd(
        nc, in_maps, core_ids=list(range(NCORES)), trace=trace
    )
    out = np.concatenate([res.results[c]["out"] for c in range(NCORES)], axis=0)
    return out.astype(np.float32), res


def kernel(**inputs):
    out, _ = run(trace=False, **inputs)
    return out



# revision 12
# speedup vs baseline: 1.3710x; 1.3710x over previous
"""Trainium2 Bass kernel for nn_BertCounterFactTransformer.

Contract: kernel(**inputs) takes FULL unsharded numpy inputs (as produced by
reference.setup_inputs()) and returns the FULL [32, 1024] float32 output.

Data-parallel over batch: 8 cores x 4 samples. Host computes masks/bounds and
packs operands; device computes gates, scores via the M-matrix identity
(S = X Wq Wk^T X^T = X M X^T), attention-weighted pooled vectors via the
gate/attn reassociation (g^T (A X) = (g^T A) X), then the MLP tail + LN.

v2 layout strategy (vs v1): all big matmuls keep the moving (rhs) operand
wide (N=256..512) and the stationary (lhsT) operand tiny so LDWEIGHTS never
bounds the PE:
  - gates: al row = wanom^T X^T (lhsT=[128,1] per k), then PE-transpose rows
    into token-partition form.
  - projections: per (type m-tile, sample-pair) rhs spans 2 samples (N=512).
  - pool: lhsT = G columns ([128,1..2]), rhs = x rows (N=512, bf16).
  - MLP tail: lhsT = fused^T tiles ([128,4]), rhs = W1/W2 row-tiles (N=512),
    so the 8.4MB of W never passes through LDWEIGHTS.
W1/W2 DMA is prefetched behind the scores phase (SBUF scoped so the
region reuse is safe), xT ships false-half first so the PE starts early.
"""

import sys

if "/opt/trn_rl_repo" not in sys.path:
    sys.path.insert(0, "/opt/trn_rl_repo")

import numpy as np
import ml_dtypes
from contextlib import ExitStack

np_bf16 = ml_dtypes.bfloat16
np_fp8 = ml_dtypes.float8_e4m3

import concourse.bacc as bacc
import concourse.bass as bass
import concourse.mybir as mybir
import concourse.tile as tile
from concourse import bass_utils

f32 = mybir.dt.float32
bf16 = mybir.dt.bfloat16
fp8 = mybir.dt.float8e4
AF = mybir.ActivationFunctionType
ALU = mybir.AluOpType

B, L, D = 32, 512, 1024
NCORES = 8
BC = B // NCORES          # samples per core
NPAIR = BC // 2
NL = L // 128             # 4 L-tiles
ND = D // 128             # 8 D-tiles
NC3 = 3 * D // 128        # 24 tiles of the 3D fused dim
SCALE = 1.0 / 32.0        # 1/sqrt(D)
OBIAS_RAW = -960.0        # -30 after * SCALE
LN_EPS = 1e-5
HALF = L // 2             # 256: false half [0,256), option half [256,512)

PROJ_NAMES = ["w_sq", "w_sk", "w_cq", "w_ck", "w_rq", "w_rk"]
PBIAS_NAMES = ["b_sq", "b_sk", "b_cq", "b_ck", "b_rq", "b_rk"]
QS, KS, QC, KC, QR, KR = range(6)
QPROJ = (QS, QC, QR)

_PROGRAM_CACHE = {}
_M_CACHE = {}


def _m_matrix(wq, wk):
    import hashlib
    wq = np.asarray(wq, dtype=np.float32)
    wk = np.asarray(wk, dtype=np.float32)
    key = hashlib.blake2b(wq.tobytes() + wk.tobytes(), digest_size=16).digest()
    if key not in _M_CACHE:
        _M_CACHE[key] = np.ascontiguousarray(wq @ wk.T)
    return _M_CACHE[key]


# ---------------------------------------------------------------------------
# fast path: requires per-slot F in {1,2} and J0 in {2,3,4}
# ---------------------------------------------------------------------------

def fast_eligible(bounds):
    return all(1 <= F <= 2 and 2 <= J0 <= NL for F, J0 in bounds)


def build_program_fast(bounds, use_fp8=False):
    """bounds[s] = (F, J0). False rows in tiles [0,F) (q-cols [0,128F)),
    option cols in [128*J0, 512). Computing a superset is harmless (masks)."""
    nc = bacc.Bacc(
        "TRN2",
        target_bir_lowering=False,
        debug=False,
        enable_asserts=False,
        num_devices=NCORES,
    )

    # per-slot geometry
    geo = []
    for s in range(BC):
        F, J0 = bounds[s]
        geo.append((F, J0, F * 128, J0 * 128, L - J0 * 128,
                    F > 0 and L - J0 * 128 > 0))
    # per-pair q-geometry (samples 2pr, 2pr+1 batched in one rhs)
    pgeo = []
    for pr in range(NPAIR):
        Fp = max(geo[2 * pr][0], geo[2 * pr + 1][0])
        pgeo.append((Fp, Fp * 128))
    FMAX = max(g[0] for g in geo)
    CQMAX = FMAX * 128

    # ---- DRAM tensors (host-packed layouts; identity DMA) ----
    xtf_d = nc.dram_tensor("xtf", [128, ND, BC, HALF], bf16,
                           kind="ExternalInput").ap()
    xto_d = nc.dram_tensor("xto", [128, ND, BC, HALF], bf16,
                           kind="ExternalInput").ap()
    xp_d = nc.dram_tensor("xp", [128, NL, BC, D], bf16,
                          kind="ExternalInput").ap()
    if use_fp8:
        m8_d = [nc.dram_tensor(f"m8_{t}", [128, ND // 2, 2, D], fp8,
                               kind="ExternalInput").ap() for t in range(3)]
        xtf8_d = nc.dram_tensor("xtf8", [128, ND // 2, 2, BC, HALF], fp8,
                                kind="ExternalInput").ap()
        deq_d = nc.dram_tensor("deq", [128, 1], f32, kind="ExternalInput").ap()
    else:
        m_d = [nc.dram_tensor(f"m_{t}", [128, ND, D], bf16,
                              kind="ExternalInput").ap() for t in range(3)]
    wanom_d = nc.dram_tensor("w_anom", [128, ND], bf16,
                             kind="ExternalInput").ap()
    fm_d = nc.dram_tensor("fmask_tp", [128, 2, BC], f32,
                          kind="ExternalInput").ap()
    ob_d = nc.dram_tensor("obias", [1, BC, L], bf16, kind="ExternalInput").ap()
    w1_d = nc.dram_tensor("w_f1", [128, NC3, D], bf16,
                          kind="ExternalInput").ap()
    w2_d = nc.dram_tensor("w_f2", [128, ND, D], bf16,
                          kind="ExternalInput").ap()
    b1r_d = nc.dram_tensor("b_f1", [1, D], bf16, kind="ExternalInput").ap()
    b2r_d = nc.dram_tensor("b_f2", [1, D], bf16, kind="ExternalInput").ap()
    lng_d = nc.dram_tensor("ln_g", [128, ND], f32, kind="ExternalInput").ap()
    lnb_d = nc.dram_tensor("ln_b", [128, ND], f32, kind="ExternalInput").ap()
    out_d = nc.dram_tensor("out", [BC, D], f32, kind="ExternalOutput").ap()

    with tile.TileContext(nc) as tc, ExitStack() as ctx:
        const_p = ctx.enter_context(tc.tile_pool(name="const", bufs=1))
        work_p = ctx.enter_context(tc.tile_pool(name="work", bufs=1))
        sm_p = ctx.enter_context(tc.tile_pool(name="small", bufs=3))
        tmp_p = ctx.enter_context(tc.tile_pool(name="tmp", bufs=2))
        ps_big = ctx.enter_context(tc.tile_pool(name="psb", bufs=4, space="PSUM"))
        ps_s = ctx.enter_context(tc.tile_pool(name="pss", bufs=4, space="PSUM"))

        es_xp = ExitStack()     # closed after scores: xtf/xto/pt
        xtf_p = es_xp.enter_context(tc.tile_pool(name="xtf", bufs=1))
        pt_p = es_xp.enter_context(tc.tile_pool(name="pt", bufs=1))
        es_w = ExitStack()      # closed right after proj: M weights (LIFO top)
        w_p = es_w.enter_context(tc.tile_pool(name="w", bufs=1))
        x_t = work_p.tile([128, NL, BC, D], bf16)   # token-partition x

        # ---- constants / small inputs (scalar queue) ----
        ones_col = const_p.tile([128, 1], f32)
        nc.vector.memset(ones_col[:], 1.0)
        ones_row = const_p.tile([1, 128], bf16)
        nc.vector.memset(ones_row[:], 1.0)
        ones14 = const_p.tile([1, BC], bf16)
        nc.vector.memset(ones14[:], 1.0)
        ones_row_f = const_p.tile([1, 128], f32)
        nc.vector.memset(ones_row_f[:], 1.0)
        iot_t = const_p.tile([128, 128], mybir.dt.int32)
        nc.gpsimd.iota(iot_t[:], pattern=[[1, 128]], base=0, channel_multiplier=-1)
        ident_f = const_p.tile([128, 128], f32)
        nc.vector.tensor_scalar(ident_f[:], iot_t[:], scalar1=0, scalar2=None,
                                op0=ALU.is_equal)
        ident_b = const_p.tile([128, 128], bf16)
        nc.vector.tensor_copy(ident_b[:], ident_f[:])

        wanom_t = const_p.tile([128, ND], bf16)
        nc.scalar.dma_start(wanom_t[:], wanom_d[:])
        fm_t = const_p.tile([128, 2, BC], f32)
        nc.scalar.dma_start(fm_t[:], fm_d[:])
        ob_t = const_p.tile([1, BC, L], bf16)
        nc.scalar.dma_start(ob_t[:], ob_d[:])
        b1r_t = const_p.tile([1, D], bf16)
        nc.scalar.dma_start(b1r_t[:], b1r_d[:])
        b2r_t = const_p.tile([1, D], bf16)
        nc.scalar.dma_start(b2r_t[:], b2r_d[:])
        lng_t = const_p.tile([128, ND], f32)
        nc.scalar.dma_start(lng_t[:], lng_d[:])
        lnb_t = const_p.tile([128, ND], f32)
        nc.scalar.dma_start(lnb_t[:], lnb_d[:])
        if use_fp8:
            deq_t = const_p.tile([128, 1], f32)
            nc.scalar.dma_start(deq_t[:], deq_d[:])

        # ---- big input DMAs ----
        xtf_t = xtf_p.tile([128, ND, BC, HALF], bf16)      # false half (q cols)
        nc.sync.dma_start(xtf_t[:], xtf_d[:])
        if use_fp8:
            xtf8_t = xtf_p.tile([128, ND // 2, 2, BC, HALF], fp8)
            nc.sync.dma_start(xtf8_t[:], xtf8_d[:])
        xto_t = xtf_p.tile([128, ND, BC, HALF], bf16)      # option half (k cols)
        nc.sync.dma_start(xto_t[:], xto_d[:])
        if use_fp8:
            wt8 = [w_p.tile([128, ND // 2, 2, D], fp8, name=f"mt8_{t}")
                   for t in range(3)]
            for t in range(3):
                nc.gpsimd.dma_start(wt8[t][:], m8_d[t][:])
        else:
            wt = [w_p.tile([128, ND, D], bf16, name=f"mt_{t}") for t in range(3)]
            for t in range(3):
                nc.gpsimd.dma_start(wt[t][:], m_d[t][:])
        nc.gpsimd.dma_start(x_t[:], xp_d[:])

        # =============== gates: al rows -> token-partition gate ===============
        # al[s, l] = x[s, l] . w_anom ; row-form matmuls, N=2*CQp per pair
        al_ps = []
        for pr in range(NPAIR):
            Fp, CQp = pgeo[pr]
            ps = ps_big.tile([1, 2, CQp], f32, tag="ps", name=f"al{pr}")
            for k in range(ND):
                nc.tensor.matmul(
                    ps[:], lhsT=wanom_t[:, k : k + 1],
                    rhs=xtf_t[:, k, 2 * pr : 2 * pr + 2, 0:CQp],
                    start=(k == 0), stop=(k == ND - 1),
                )
            al_ps.append(ps)
        al_sb = [work_p.tile([1, 2, CQMAX], bf16, name=f"alsb{pr}")
                 for pr in range(NPAIR)]
        for pr in range(NPAIR):
            Fp, CQp = pgeo[pr]
            nc.scalar.copy(al_sb[pr][:, :, 0:CQp], al_ps[pr][:])

        # transpose [1,128] row chunks -> [128,1] token-partition cols
        gcol_t = work_p.tile([128, 2, BC], f32)            # (tile, sample)
        nc.vector.memset(gcol_t[:], 0.0)
        for pr in range(NPAIR):
            for j in range(2):
                s = 2 * pr + j
                for t in range(geo[s][0]):
                    tr = ps_s.tile([128, 1], bf16, tag="pss", name="gtr")
                    nc.tensor.transpose(
                        tr[:], al_sb[pr][:, j, t * 128 : (t + 1) * 128],
                        ident_b[0:1, 0:1],
                    )
                    nc.vector.tensor_copy(gcol_t[:, t, s : s + 1], tr[:])

        # ghat = exp(al) * fmask  (token-partition, all samples at once)
        eg_t = work_p.tile([128, 2, BC], f32)
        nc.scalar.activation(eg_t[:], gcol_t[:], AF.Exp)
        ghat_t = work_p.tile([128, 2, BC], f32)
        nc.vector.tensor_mul(ghat_t[:], eg_t[:], fm_t[:])
        gsum_t = work_p.tile([128, BC], f32)
        for s in range(BC):
            nc.vector.tensor_reduce(
                gsum_t[:, s : s + 1], ghat_t[:, 0 : geo[s][0], s],
                axis=mybir.AxisListType.X, op=ALU.add,
            )
        S_ps = ps_s.tile([1, BC], f32, tag="pss", name="S")
        nc.tensor.matmul(S_ps[:], lhsT=ones_col[:], rhs=gsum_t[:],
                         start=True, stop=True)
        Smax_t = sm_p.tile([1, BC], f32, tag="Smax")
        nc.vector.tensor_scalar_max(Smax_t[:], S_ps[:], 1e-8)
        recipS_t = sm_p.tile([1, BC], f32, tag="recipS")
        nc.vector.reciprocal(recipS_t[:], Smax_t[:])
        rb_ps = ps_s.tile([128, BC], f32, tag="pss", name="rb")
        nc.tensor.matmul(rb_ps[:], lhsT=ones_row_f[:], rhs=recipS_t[:],
                         start=True, stop=True)
        rb_t = work_p.tile([128, BC], f32)
        nc.vector.tensor_copy(rb_t[:], rb_ps[:])
        # G3[:, lt, :, s] = [gate, r_rep, r_sup] columns for the pool chains
        G3_t = work_p.tile([128, NL, 3, BC], bf16)
        nc.vector.memset(G3_t[:], 0.0)
        gate_t = work_p.tile([128, 2, BC], bf16)
        for s in range(BC):
            nc.vector.tensor_scalar_mul(
                gate_t[:, :, s], ghat_t[:, :, s], rb_t[:, s : s + 1]
            )
            nc.vector.tensor_copy(G3_t[:, 0:2, 0, s], gate_t[:, :, s])

        # =============== projections: pt = (X_f M)^T  [d-part, q-cols] =======
        pt_t = pt_p.tile([128, 3, ND, BC, CQMAX], bf16)
        ci = 0
        for t in range(3):
            for m in range(ND):
                for pr in range(NPAIR):
                    Fp, CQp = pgeo[pr]
                    ps = ps_big.tile([128, 2, CQp], f32, tag="ps", name="proj")
                    if use_fp8:
                        for j in range(2):
                            for kp in range(ND // 2):
                                nc.tensor.matmul(
                                    ps[:, j, :],
                                    lhsT=wt8[t][:, kp, :, m * 128 : (m + 1) * 128],
                                    rhs=xtf8_t[:, kp, :, 2 * pr + j, 0:CQp],
                                    start=(kp == 0), stop=(kp == ND // 2 - 1),
                                    perf_mode=mybir.MatmulPerfMode.DoubleRow,
                                )
                    else:
                        for k in range(ND):
                            nc.tensor.matmul(
                                ps[:],
                                lhsT=wt[t][:, k, m * 128 : (m + 1) * 128],
                                rhs=xtf_t[:, k, 2 * pr : 2 * pr + 2, 0:CQp],
                                start=(k == 0), stop=(k == ND - 1),
                            )
                    dst = pt_t[:, t, m, 2 * pr : 2 * pr + 2, 0:CQp]
                    if use_fp8:
                        if ci % 2 == 0:
                            nc.vector.tensor_scalar_mul(dst, ps[:], deq_t[:, 0:1])
                        else:
                            nc.scalar.mul(dst, ps[:], deq_t[:, 0:1])
                    else:
                        if ci % 2 == 0:
                            nc.vector.tensor_copy(dst, ps[:])
                        else:
                            nc.scalar.copy(dst, ps[:])
                    ci += 1

        es_w.close()   # pop M region (LIFO top)

        # =============== scores -> E (bf16) + coeffs =========================
        E_t = work_p.tile([128, 2, 2, BC, HALF], bf16)   # (type: rep,sup; it)
        co_t = work_p.tile([128, 2, 2, BC], bf16)        # (type, it, s)
        for s in range(BC):
            F, J0, CQ, OJ, NO, have_attn = geo[s]
            if not have_attn:
                continue
            for it in range(F):
                isl = slice(it * 128, (it + 1) * 128)
                ps_sup = ps_big.tile([128, NO], f32, tag="ps", name="psup")
                for k in range(ND):
                    nc.tensor.matmul(
                        ps_sup[:], lhsT=pt_t[:, 0, k, s, isl],
                        rhs=xto_t[:, k, s, OJ - HALF : HALF],
                        start=(k == 0), stop=False,
                    )
                nc.tensor.matmul(ps_sup[:], lhsT=ones_row[:],
                                 rhs=ob_t[0:1, s, OJ:L],
                                 start=False, stop=True)
                ps_con = ps_big.tile([128, NO], f32, tag="ps", name="pcon")
                for k in range(ND):
                    nc.tensor.matmul(
                        ps_con[:], lhsT=pt_t[:, 1, k, s, isl],
                        rhs=xto_t[:, k, s, OJ - HALF : HALF],
                        start=(k == 0), stop=(k == ND - 1),
                    )
                ps_rep = ps_big.tile([128, NO], f32, tag="ps", name="prep")
                for k in range(ND):
                    nc.tensor.matmul(
                        ps_rep[:], lhsT=pt_t[:, 2, k, s, isl],
                        rhs=xto_t[:, k, s, OJ - HALF : HALF],
                        start=(k == 0), stop=False,
                    )
                nc.tensor.matmul(ps_rep[:], lhsT=ones_row[:],
                                 rhs=ob_t[0:1, s, OJ:L],
                                 start=False, stop=True)

                T_t = tmp_p.tile([128, NO], f32, tag="T")
                nc.scalar.activation(T_t[:], ps_con[:], AF.Tanh, scale=SCALE)
                A_t = tmp_p.tile([128, NO], f32, tag="A")
                nc.vector.scalar_tensor_tensor(
                    A_t[:], in0=ps_rep[:], scalar=SCALE, in1=T_t[:],
                    op0=ALU.mult, op1=ALU.add,
                )
                rs_sup = sm_p.tile([128, 1], f32, tag="rssup")
                nc.scalar.activation(E_t[:, 1, it, s, 0:NO], ps_sup[:], AF.Exp,
                                     scale=SCALE, accum_out=rs_sup[:])
                rs_rep = sm_p.tile([128, 1], f32, tag="rsrep")
                nc.scalar.activation(E_t[:, 0, it, s, 0:NO], A_t[:], AF.Exp,
                                     accum_out=rs_rep[:])
                rc_sup = sm_p.tile([128, 1], f32, tag="rcsup")
                nc.vector.reciprocal(rc_sup[:], rs_sup[:])
                nc.vector.tensor_mul(co_t[:, 1, it, s : s + 1],
                                     gate_t[:, it, s : s + 1], rc_sup[:])
                rc_rep = sm_p.tile([128, 1], f32, tag="rcrep")
                nc.vector.reciprocal(rc_rep[:], rs_rep[:])
                nc.vector.tensor_mul(co_t[:, 0, it, s : s + 1],
                                     gate_t[:, it, s : s + 1], rc_rep[:])

        es_xp.close()  # xtf/xto/pt region free -> W1/W2 land there
        tail_p = ctx.enter_context(tc.tile_pool(name="tail", bufs=1))
        w1_t = tail_p.tile([128, NC3, D], bf16)
        nc.scalar.dma_start(w1_t[:], w1_d[:])
        w2_t = tail_p.tile([128, ND, D], bf16)
        nc.scalar.dma_start(w2_t[:], w2_d[:])

        # =============== r rows -> token-partition cols of G3 ================
        # r[type, :] = sum_it co[type,it]^T E[type,it,:]   (row form, N=NO)
        rsb_t = [work_p.tile([1, BC, HALF], bf16, name=f"rsb{ty}")
                 for ty in range(2)]
        for s in range(BC):
            F, J0, CQ, OJ, NO, have_attn = geo[s]
            if not have_attn:
                continue
            for ty in range(2):
                r_ps = ps_s.tile([1, HALF], f32, tag="pss", name="rps")
                for it in range(F):
                    nc.tensor.matmul(
                        r_ps[:, 0:NO], lhsT=co_t[:, ty, it, s : s + 1],
                        rhs=E_t[:, ty, it, s, 0:NO],
                        start=(it == 0), stop=(it == F - 1),
                    )
                nc.scalar.copy(rsb_t[ty][:, s, 0:NO], r_ps[:, 0:NO])
            for ty in range(2):
                for jt in range(J0, NL):
                    off = jt * 128 - OJ
                    tr = ps_s.tile([128, 1], bf16, tag="pss", name="rtr")
                    nc.tensor.transpose(
                        tr[:], rsb_t[ty][:, s, off : off + 128],
                        ident_b[0:1, 0:1],
                    )
                    nc.vector.tensor_copy(G3_t[:, jt, 1 + ty, s : s + 1],
                                          tr[:])

        # =============== pool: P_s = [anom; rep; sup] rows [3, 1024] =========
        P_sb = work_p.tile([3, BC, D], bf16)    # (type, sample, d)
        for s in range(BC):
            for h in range(2):
                hs = slice(h * 512, (h + 1) * 512)
                p_ps = ps_big.tile([3, 512], f32, tag="ps", name="pps")
                for lt in range(NL):
                    nc.tensor.matmul(
                        p_ps[:], lhsT=G3_t[:, lt, :, s],
                        rhs=x_t[:, lt, s, hs],
                        start=(lt == 0), stop=(lt == NL - 1),
                    )
                nc.scalar.copy(P_sb[:, s, hs], p_ps[:])

        # fused^T tiles: [128 (3072-chunk kt = t*8+m), samples]
        fuT_t = tail_p.tile([128, NC3, BC], bf16)
        for s in range(BC):
            for m in range(ND):
                tr = ps_s.tile([128, 3], bf16, tag="pss", name="futr")
                nc.tensor.transpose(
                    tr[:], P_sb[:, s, m * 128 : (m + 1) * 128],
                    ident_b[0:3, 0:3],
                )
                for t in range(3):
                    nc.vector.tensor_copy(
                        fuT_t[:, t * ND + m, s : s + 1], tr[:, t : t + 1]
                    )

        # =============== MLP tail (row form, W moving) =======================
        h_ps = [ps_big.tile([BC, 512], f32, tag="ps", name=f"hps{h}")
                for h in range(2)]
        for h in range(2):
            hs = slice(h * 512, (h + 1) * 512)
            nc.tensor.matmul(h_ps[h][:], lhsT=ones14[:], rhs=b1r_t[:, hs],
                             start=True, stop=False)
            for k in range(NC3):
                nc.tensor.matmul(
                    h_ps[h][:], lhsT=fuT_t[:, k, :], rhs=w1_t[:, k, hs],
                    start=False, stop=(k == NC3 - 1),
                )
        hrow_t = work_p.tile([BC, D], bf16)
        for h in range(2):
            hs = slice(h * 512, (h + 1) * 512)
            nc.scalar.activation(hrow_t[:, hs], h_ps[h][:], AF.Relu)
        hT_t = tail_p.tile([128, ND, BC], bf16)
        for m in range(ND):
            tr = ps_s.tile([128, BC], bf16, tag="pss", name="htr")
            nc.tensor.transpose(
                tr[:], hrow_t[:, m * 128 : (m + 1) * 128], ident_b[0:BC, 0:BC]
            )
            nc.vector.tensor_copy(hT_t[:, m, :], tr[:])

        y_ps = [ps_big.tile([BC, 512], f32, tag="ps", name=f"yps{h}")
                for h in range(2)]
        for h in range(2):
            hs = slice(h * 512, (h + 1) * 512)
            nc.tensor.matmul(y_ps[h][:], lhsT=ones14[:], rhs=b2r_t[:, hs],
                             start=True, stop=False)
            for k in range(ND):
                nc.tensor.matmul(
                    y_ps[h][:], lhsT=hT_t[:, k, :], rhs=w2_t[:, k, hs],
                    start=False, stop=(k == ND - 1),
                )
        yrow_t = work_p.tile([BC, D], f32)
        for h in range(2):
            hs = slice(h * 512, (h + 1) * 512)
            nc.scalar.copy(yrow_t[:, hs], y_ps[h][:])

        # =============== LayerNorm (column form) =============================
        yT_t = tail_p.tile([128, ND, BC], f32)
        sq_t = tail_p.tile([128, ND, BC], f32)
        for m in range(ND):
            tr = ps_s.tile([128, BC], f32, tag="pss", name="ytr")
            nc.tensor.transpose(
                tr[:], yrow_t[:, m * 128 : (m + 1) * 128], ident_f[0:BC, 0:BC]
            )
            nc.vector.tensor_copy(yT_t[:, m, :], tr[:])
            nc.scalar.square(sq_t[:, m, :], yT_t[:, m, :])

        sum_ps = ps_s.tile([1, BC], f32, tag="pss", name="sums")
        for m in range(ND):
            nc.tensor.matmul(sum_ps[:], lhsT=ones_col[:], rhs=yT_t[:, m, :],
                             start=(m == 0), stop=(m == ND - 1))
        ssq_ps = ps_s.tile([1, BC], f32, tag="pss", name="ssq")
        for m in range(ND):
            nc.tensor.matmul(ssq_ps[:], lhsT=ones_col[:], rhs=sq_t[:, m, :],
                             start=(m == 0), stop=(m == ND - 1))
        mean_t = sm_p.tile([1, BC], f32, tag="mean")
        nc.scalar.mul(mean_t[:], sum_ps[:], 1.0 / D)
        msq_t = sm_p.tile([1, BC], f32, tag="msq")
        nc.scalar.mul(msq_t[:], ssq_ps[:], 1.0 / D)
        m2_t = sm_p.tile([1, BC], f32, tag="m2")
        nc.vector.tensor_mul(m2_t[:], mean_t[:], mean_t[:])
        var_t = sm_p.tile([1, BC], f32, tag="var")
        nc.vector.tensor_sub(var_t[:], msq_t[:], m2_t[:])
        nc.vector.tensor_scalar_add(var_t[:], var_t[:], LN_EPS)
        sd_t = sm_p.tile([1, BC], f32, tag="sd")
        nc.scalar.sqrt(sd_t[:], var_t[:])
        rstd_t = sm_p.tile([1, BC], f32, tag="rstd")
        nc.vector.reciprocal(rstd_t[:], sd_t[:])

        mb_ps = ps_s.tile([128, BC], f32, tag="pss", name="mb")
        nc.tensor.matmul(mb_ps[:], lhsT=ones_row_f[:], rhs=mean_t[:],
                         start=True, stop=True)
        mb_t = sm_p.tile([128, BC], f32, tag="mbt")
        nc.vector.tensor_copy(mb_t[:], mb_ps[:])
        rb2_ps = ps_s.tile([128, BC], f32, tag="pss", name="rb2")
        nc.tensor.matmul(rb2_ps[:], lhsT=ones_row_f[:], rhs=rstd_t[:],
                         start=True, stop=True)
        rb2_t = sm_p.tile([128, BC], f32, tag="rb2t")
        nc.vector.tensor_copy(rb2_t[:], rb2_ps[:])

        zrow_t = tail_p.tile([BC, D], f32)
        for m in range(ND):
            z_t = tmp_p.tile([128, BC], f32, tag="z")
            nc.vector.tensor_sub(z_t[:], yT_t[:, m, :], mb_t[:])
            nc.vector.tensor_mul(z_t[:], z_t[:], rb2_t[:])
            z2_t = tmp_p.tile([128, BC], f32, tag="z2")
            nc.vector.tensor_scalar(
                z2_t[:], z_t[:], scalar1=lng_t[:, m : m + 1],
                scalar2=lnb_t[:, m : m + 1], op0=ALU.mult, op1=ALU.add,
            )
            tr_ps = ps_s.tile([BC, 128], f32, tag="pss", name="ztr")
            nc.tensor.transpose(tr_ps[:], z2_t[:], ident_f[:])
            nc.vector.tensor_copy(zrow_t[:, m * 128 : (m + 1) * 128], tr_ps[:])
        nc.sync.dma_start(out_d[:, :], zrow_t[:, :])

    nc.compile()
    return nc


def _host_prep_fast(inputs, fmask, obias, bounds, use_fp8=False):
    x = np.asarray(inputs["x"], dtype=np.float32)

    def w(name):
        return np.ascontiguousarray(np.asarray(inputs[name], dtype=np.float32))

    def ppart(name):
        return np.ascontiguousarray(
            np.asarray(inputs[name], dtype=np.float32).reshape(ND, 128).T)

    shared = {}
    Ms = [_m_matrix(inputs[qn], inputs[kn])
          for qn, kn in (("w_sq", "w_sk"), ("w_cq", "w_ck"), ("w_rq", "w_rk"))]
    SX = None
    if use_fp8:
        # one global scale per side; dequant applied on PSUM readout
        SX = 200.0 / max(float(np.abs(x).max()), 1e-30)
        SM = min(200.0 / max(float(np.abs(M).max()), 1e-30) for M in Ms)
        for t, M in enumerate(Ms):
            a = (M * SM).reshape(ND // 2, 2, 128, D).transpose(2, 0, 1, 3)
            shared[f"m8_{t}"] = np.ascontiguousarray(a).astype(np_fp8)
        shared["deq"] = np.full((128, 1), 1.0 / (SM * SX), np.float32)
    else:
        for t, M in enumerate(Ms):
            a = M.reshape(ND, 128, D).transpose(1, 0, 2)
            shared[f"m_{t}"] = np.ascontiguousarray(a).astype(np_bf16)

    shared["w_anom"] = np.ascontiguousarray(
        w("w_anom").reshape(ND, 128).T).astype(np_bf16)
    shared["w_f1"] = np.ascontiguousarray(
        w("w_f1").reshape(NC3, 128, D).transpose(1, 0, 2)).astype(np_bf16)
    shared["w_f2"] = np.ascontiguousarray(
        w("w_f2").reshape(ND, 128, D).transpose(1, 0, 2)).astype(np_bf16)
    shared["b_f1"] = w("b_f1").reshape(1, D).astype(np_bf16)
    shared["b_f2"] = w("b_f2").reshape(1, D).astype(np_bf16)
    shared["ln_g"] = ppart("ln_g")
    shared["ln_b"] = ppart("ln_b")

    in_maps = []
    for c in range(NCORES):
        sl = slice(c * BC, (c + 1) * BC)
        xc = x[sl]                                   # [BC, L, D]
        m = dict(shared)
        xf = xc[:, :HALF, :]                         # [BC, 256, D]
        xo = xc[:, HALF:, :]
        # xtf[p, k, s, l] = x[s, l, 128k+p]
        m["xtf"] = np.ascontiguousarray(
            xf.transpose(2, 0, 1).reshape(ND, 128, BC, HALF)
            .transpose(1, 0, 2, 3)).astype(np_bf16)
        m["xto"] = np.ascontiguousarray(
            xo.transpose(2, 0, 1).reshape(ND, 128, BC, HALF)
            .transpose(1, 0, 2, 3)).astype(np_bf16)
        if use_fp8:
            a = (xf * SX).transpose(2, 0, 1).reshape(ND // 2, 2, 128, BC, HALF)
            m["xtf8"] = np.ascontiguousarray(
                a.transpose(2, 0, 1, 3, 4)).astype(np_fp8)
        # xp[p, lt, s, d] = x[s, 128*lt+p, d]
        m["xp"] = np.ascontiguousarray(
            xc.reshape(BC, NL, 128, D).transpose(2, 1, 0, 3)).astype(np_bf16)
        fm = fmask[sl][:, : 2 * 128].reshape(BC, 2, 128).transpose(2, 1, 0)
        m["fmask_tp"] = np.ascontiguousarray(fm).astype(np.float32)
        m["obias"] = np.ascontiguousarray(obias[sl].reshape(1, BC, L)).astype(np_bf16)
        in_maps.append(m)
    return in_maps


# ---------------------------------------------------------------------------
# generic fallback (v1 baseline program)
# ---------------------------------------------------------------------------

def build_program(bounds=((2, 2),) * BC, use_m=True, enable_asserts=False):
    """bounds[s] = (F, J0): false rows live in tiles [0,F), option cols in
    [128*J0, 512). Computing a superset is always correct (masks zero it)."""
    nc = bacc.Bacc(
        "TRN2",
        target_bir_lowering=False,
        debug=False,
        enable_asserts=enable_asserts,
        num_devices=NCORES,
    )

    xT_d = nc.dram_tensor("xT", [BC, D, L], bf16, kind="ExternalInput").ap()
    x_d = nc.dram_tensor("x", [BC, L, D], f32, kind="ExternalInput").ap()
    fmask_d = nc.dram_tensor("fmask", [BC, L], f32, kind="ExternalInput").ap()
    obias_d = nc.dram_tensor("obias", [BC, L], bf16, kind="ExternalInput").ap()

    if use_m:
        W_d = {p: nc.dram_tensor(n, [D, D], bf16, kind="ExternalInput").ap()
               for p, n in ((QS, "m_sup"), (QC, "m_con"), (QR, "m_rep"))}
    else:
        W_d = {p: nc.dram_tensor(PROJ_NAMES[p], [D, D], bf16, kind="ExternalInput").ap()
               for p in range(6)}
    Brow_d = {} if use_m else {
        p: nc.dram_tensor(PBIAS_NAMES[p], [1, D], bf16, kind="ExternalInput").ap()
        for p in range(6)}
    wanom_d = nc.dram_tensor("w_anom", [D, 1], bf16, kind="ExternalInput").ap()
    wf1_d = nc.dram_tensor("w_f1", [ND, 128, NC3 * 128], bf16, kind="ExternalInput").ap()
    wf2_d = nc.dram_tensor("w_f2", [ND, 128, ND * 128], bf16, kind="ExternalInput").ap()
    bf1_d = nc.dram_tensor("b_f1", [128, ND], f32, kind="ExternalInput").ap()
    bf2_d = nc.dram_tensor("b_f2", [128, ND], f32, kind="ExternalInput").ap()
    lng_d = nc.dram_tensor("ln_g", [128, ND], f32, kind="ExternalInput").ap()
    lnb_d = nc.dram_tensor("ln_b", [128, ND], f32, kind="ExternalInput").ap()

    out_d = nc.dram_tensor("out", [BC, D], f32, kind="ExternalOutput").ap()

    with tile.TileContext(nc) as tc, ExitStack() as ctx:
        const_p = ctx.enter_context(tc.tile_pool(name="const", bufs=1))
        tmp_p = ctx.enter_context(tc.tile_pool(name="tmp", bufs=2))
        sm_p = ctx.enter_context(tc.tile_pool(name="small", bufs=3))
        tail_p = ctx.enter_context(tc.tile_pool(name="tail", bufs=1))
        ps_big = ctx.enter_context(tc.tile_pool(name="psb", bufs=4, space="PSUM"))
        ps_s = ctx.enter_context(tc.tile_pool(name="pss", bufs=4, space="PSUM"))
        es2 = ExitStack()   # closed after phase C: x, E
        x_p = es2.enter_context(tc.tile_pool(name="x", bufs=3))
        e_p = es2.enter_context(tc.tile_pool(name="emat", bufs=2))
        es1 = ExitStack()   # closed after phase B: xT, W, proj
        xT_p = es1.enter_context(tc.tile_pool(name="xT", bufs=1))
        w_p = es1.enter_context(tc.tile_pool(name="w", bufs=2))
        proj_p = es1.enter_context(tc.tile_pool(name="proj", bufs=1))

        # ---- constants ----
        ones_row = const_p.tile([1, L], bf16)
        nc.vector.memset(ones_row[:], 1.0)
        ones_f = const_p.tile([1, 128], f32)
        nc.vector.memset(ones_f[:], 1.0)
        ones_col = const_p.tile([128, 1], f32)
        nc.vector.memset(ones_col[:], 1.0)
        iot_t = const_p.tile([128, 128], mybir.dt.int32)
        nc.gpsimd.iota(iot_t[:], pattern=[[1, 128]], base=0, channel_multiplier=-1)
        ident_t = const_p.tile([128, 128], f32)
        nc.vector.tensor_scalar(ident_t[:], iot_t[:], scalar1=0, scalar2=None,
                                op0=ALU.is_equal)

        wanom_t = const_p.tile([128, ND], bf16)
        nc.scalar.dma_start(wanom_t[:], wanom_d[:, 0].rearrange("(k p) -> p k", p=128))
        brow_t = {}
        for p in Brow_d:
            brow_t[p] = const_p.tile([1, D], bf16, name=f"brow{p}")
            nc.sync.dma_start(brow_t[p][:], Brow_d[p][:])
        bf1_t = const_p.tile([128, ND], f32)
        nc.scalar.dma_start(bf1_t[:], bf1_d[:])
        bf2_t = const_p.tile([128, ND], f32)
        nc.scalar.dma_start(bf2_t[:], bf2_d[:])
        lng_t = const_p.tile([128, ND], f32)
        nc.scalar.dma_start(lng_t[:], lng_d[:])
        lnb_t = const_p.tile([128, ND], f32)
        nc.scalar.dma_start(lnb_t[:], lnb_d[:])

        fusedT = tail_p.tile([128, NC3, BC], bf16)

        # per-slot geometry
        geo = []
        for s in range(BC):
            F, J0 = bounds[s]
            geo.append((F, J0, F * 128, J0 * 128, L - J0 * 128,
                        F > 0 and L - J0 * 128 > 0))

        # ---- Phase A: xT resident + gates; M weights via one DMA each ----
        xT_t = xT_p.tile([128, BC * ND, L], bf16)
        fm_ts, ob_ts, x_ts = [], [], []
        for s in range(BC):
            nc.sync.dma_start(
                xT_t[:, s * ND : (s + 1) * ND, :],
                xT_d[s].rearrange("(k p) i -> p k i", p=128),
            )
            fm_t = sm_p.tile([128, NL], f32, tag="fm", bufs=BC, name=f"fm{s}")
            nc.scalar.dma_start(fm_t[:], fmask_d[s].rearrange("(t p) -> p t", p=128))
            fm_ts.append(fm_t)
            ob_t = sm_p.tile([1, L], bf16, tag="ob", bufs=2, name=f"ob{s}")
            nc.scalar.dma_start(ob_t[:], obias_d[s : s + 1, :])
            ob_ts.append(ob_t)

        gate_ts = []
        for s in range(BC):
            F, J0, CQ, OJ, NO, have_attn = geo[s]
            gate_t = sm_p.tile([128, NL], f32, tag="gate", bufs=BC, name=f"gate{s}")
            gate_ts.append(gate_t)
            if F == 0:
                continue
            ghat_t = sm_p.tile([128, NL], f32, tag="ghat")
            for it in range(F):
                al_ps = ps_s.tile([128, 1], f32, tag="pss")
                for k in range(ND):
                    nc.tensor.matmul(
                        al_ps[:],
                        lhsT=xT_t[:, s * ND + k, it * 128 : (it + 1) * 128],
                        rhs=wanom_t[:, k : k + 1],
                        start=(k == 0), stop=(k == ND - 1),
                    )
                eg_t = sm_p.tile([128, 1], f32, tag="eg")
                nc.scalar.activation(eg_t[:], al_ps[:], AF.Exp)
                nc.vector.tensor_mul(
                    ghat_t[:, it : it + 1], eg_t[:], fm_ts[s][:, it : it + 1]
                )
            gsum_t = sm_p.tile([128, 1], f32, tag="gsum")
            nc.vector.tensor_reduce(
                gsum_t[:], ghat_t[:, 0:F], axis=mybir.AxisListType.X, op=ALU.add
            )
            S_ps = ps_s.tile([1, 1], f32, tag="pss")
            nc.tensor.matmul(S_ps[:], lhsT=gsum_t[:], rhs=ones_col[:],
                             start=True, stop=True)
            Smax_t = sm_p.tile([1, 1], f32, tag="Smax")
            nc.vector.tensor_scalar_max(Smax_t[:], S_ps[:], 1e-8)
            Sb_ps = ps_s.tile([128, 1], f32, tag="pss")
            nc.tensor.matmul(Sb_ps[:], lhsT=ones_f[:], rhs=Smax_t[:],
                             start=True, stop=True)
            recipS_t = sm_p.tile([128, 1], f32, tag="recipS")
            nc.vector.reciprocal(recipS_t[:], Sb_ps[:])
            nc.vector.tensor_scalar_mul(gate_t[:, 0:F], ghat_t[:, 0:F],
                                        recipS_t[:])

        # ---- projections: one gpsimd DMA per M matrix, all samples inner ----
        projs = [[None] * BC for _ in range(6)]
        proj_list = list(QPROJ) if use_m else list(range(6))
        for p in proj_list:
            qside = p in QPROJ
            widths = [
                ((g[2] if qside else g[4]) if g[5] else 0) for g in geo
            ]
            wmax = max(widths)
            if wmax == 0:
                continue
            wt = w_p.tile([128, ND, D], bf16, tag="w", name=f"w{p}")
            nc.gpsimd.dma_start(wt[:], W_d[p].rearrange("(k p) c -> p k c", p=128))
            pt = proj_p.tile([128, BC, ND, wmax], bf16, tag=f"proj{p}")
            for m in range(ND):
                for s in range(BC):
                    width = widths[s]
                    if width == 0:
                        continue
                    lo = 0 if qside else geo[s][3]
                    ps = ps_big.tile([128, width], f32, tag="ps")
                    for k in range(ND):
                        nc.tensor.matmul(
                            ps[:], lhsT=wt[:, k, m * 128 : (m + 1) * 128],
                            rhs=xT_t[:, s * ND + k, lo : lo + width],
                            start=(k == 0), stop=(use_m and k == ND - 1),
                        )
                    if not use_m:
                        nc.tensor.matmul(
                            ps[:], lhsT=brow_t[p][:, m * 128 : (m + 1) * 128],
                            rhs=ones_row[:, 0:width], start=False, stop=True,
                        )
                    nc.vector.tensor_copy(pt[:, s, m, :], ps[:])
            for s in range(BC):
                if widths[s]:
                    projs[p][s] = pt

        for s in range(BC):
            x_t = x_p.tile([128, NL, D], f32, tag="x", name=f"x{s}")
            nc.sync.dma_start(x_t[:], x_d[s].rearrange("(t p) d -> p t d", p=128))
            x_ts.append(x_t)

        # ---- Phase B: scores -> E, coeffs (all samples) ----
        E_sups, E_reps, co_sups, co_reps = {}, {}, {}, {}
        for s in range(BC):
            F, J0, CQ, OJ, NO, have_attn = geo[s]
            if not have_attn:
                continue
            E_sup = e_p.tile([128, max(F, 1), NO], f32, tag="esup", bufs=BC,
                             name=f"esup{s}")
            E_rep = e_p.tile([128, max(F, 1), NO], f32, tag="erep", bufs=BC,
                             name=f"erep{s}")
            co_sup = sm_p.tile([128, NL], f32, tag="cosup", bufs=BC,
                               name=f"cosup{s}")
            co_rep = sm_p.tile([128, NL], f32, tag="corep", bufs=BC,
                               name=f"corep{s}")
            E_sups[s], E_reps[s] = E_sup, E_rep
            co_sups[s], co_reps[s] = co_sup, co_rep
            gate_t = gate_ts[s]
            ob_t = ob_ts[s]
            for it in range(F):
                isl = slice(it * 128, (it + 1) * 128)
                ps_sup = ps_big.tile([128, NO], f32, tag="ps")
                for k in range(ND):
                    nc.tensor.matmul(
                        ps_sup[:], lhsT=projs[QS][s][:, s, k, isl],
                        rhs=(xT_t[:, s * ND + k, OJ:L] if use_m
                             else projs[KS][s][:, s, k, 0:NO]),
                        start=(k == 0), stop=False,
                    )
                nc.tensor.matmul(ps_sup[:], lhsT=ones_row[:, 0:128],
                                 rhs=ob_t[:, OJ:L], start=False, stop=True)
                ps_con = ps_big.tile([128, NO], f32, tag="ps")
                for k in range(ND):
                    nc.tensor.matmul(
                        ps_con[:], lhsT=projs[QC][s][:, s, k, isl],
                        rhs=(xT_t[:, s * ND + k, OJ:L] if use_m
                             else projs[KC][s][:, s, k, 0:NO]),
                        start=(k == 0), stop=(k == ND - 1),
                    )
                ps_rep = ps_big.tile([128, NO], f32, tag="ps")
                for k in range(ND):
                    nc.tensor.matmul(
                        ps_rep[:], lhsT=projs[QR][s][:, s, k, isl],
                        rhs=(xT_t[:, s * ND + k, OJ:L] if use_m
                             else projs[KR][s][:, s, k, 0:NO]),
                        start=(k == 0), stop=False,
                    )
                nc.tensor.matmul(ps_rep[:], lhsT=ones_row[:, 0:128],
                                 rhs=ob_t[:, OJ:L], start=False, stop=True)

                T_t = tmp_p.tile([128, NO], f32, tag="T")
                nc.scalar.activation(T_t[:], ps_con[:], AF.Tanh, scale=SCALE)
                A_t = tmp_p.tile([128, NO], f32, tag="A")
                nc.vector.scalar_tensor_tensor(
                    A_t[:], in0=ps_rep[:], scalar=SCALE, in1=T_t[:],
                    op0=ALU.mult, op1=ALU.add,
                )
                rs_sup = sm_p.tile([128, 1], f32, tag="rssup")
                nc.scalar.activation(E_sup[:, it, :], ps_sup[:], AF.Exp,
                                     scale=SCALE, accum_out=rs_sup[:])
                rs_rep = sm_p.tile([128, 1], f32, tag="rsrep")
                nc.scalar.activation(E_rep[:, it, :], A_t[:], AF.Exp,
                                     accum_out=rs_rep[:])
                rc_sup = sm_p.tile([128, 1], f32, tag="rcsup")
                nc.vector.reciprocal(rc_sup[:], rs_sup[:])
                nc.vector.tensor_mul(co_sup[:, it : it + 1],
                                     gate_t[:, it : it + 1], rc_sup[:])
                rc_rep = sm_p.tile([128, 1], f32, tag="rcrep")
                nc.vector.reciprocal(rc_rep[:], rs_rep[:])
                nc.vector.tensor_mul(co_rep[:, it : it + 1],
                                     gate_t[:, it : it + 1], rc_rep[:])

        es1.close()

        # ---- Phase C: r vectors, G, pooled (all samples) ----
        for s in range(BC):
            F, J0, CQ, OJ, NO, have_attn = geo[s]
            x_t = x_ts[s]

            G_t = sm_p.tile([128, NL, 3], f32, tag="G")
            nc.vector.memset(G_t[:], 0.0)
            if F > 0:
                for it in range(F):
                    nc.vector.tensor_copy(G_t[:, it, 0:1],
                                          gate_ts[s][:, it : it + 1])
            if have_attn:
                E_sup, E_rep = E_sups[s], E_reps[s]
                co_sup, co_rep = co_sups[s], co_reps[s]
                for jt in range(J0, NL):
                    jsl = slice(jt * 128 - OJ, jt * 128 - OJ + 128)
                    r_ps = ps_s.tile([128, 2], f32, tag="pss")
                    for it in range(F):
                        nc.tensor.matmul(
                            r_ps[:, 0:1], lhsT=E_rep[:, it, jsl],
                            rhs=co_rep[:, it : it + 1],
                            start=(it == 0), stop=(it == F - 1),
                        )
                    for it in range(F):
                        nc.tensor.matmul(
                            r_ps[:, 1:2], lhsT=E_sup[:, it, jsl],
                            rhs=co_sup[:, it : it + 1],
                            start=(it == 0), stop=(it == F - 1),
                        )
                    nc.vector.tensor_copy(G_t[:, jt, 1:3], r_ps[:, 0:2])

            rts = sorted(set(range(F)) | (set(range(J0, NL)) if have_attn else set()))
            if not rts:
                rts = [0]
            for m in range(ND):
                pool_ps = ps_s.tile([128, 3], f32, tag="pss")
                for i, rt in enumerate(rts):
                    nc.tensor.matmul(
                        pool_ps[:], lhsT=x_t[:, rt, m * 128 : (m + 1) * 128],
                        rhs=G_t[:, rt, :],
                        start=(i == 0), stop=(i == len(rts) - 1),
                    )
                for t in range(3):
                    nc.vector.tensor_copy(
                        fusedT[:, t * ND + m, s : s + 1], pool_ps[:, t : t + 1]
                    )

        es2.close()

        # ---- batched MLP tail ----
        wf1_p = ctx.enter_context(tc.tile_pool(name="wf1", bufs=8))
        hT_t = tail_p.tile([128, ND, BC], bf16)
        for m in range(ND):
            wt = wf1_p.tile([128, NC3, 128], bf16, tag="wf1")
            nc.gpsimd.dma_start(wt[:], wf1_d[m].rearrange("p (k c) -> p k c", c=128))
            h_ps = ps_s.tile([128, BC], f32, tag="pss")
            for k in range(NC3):
                nc.tensor.matmul(h_ps[:], lhsT=wt[:, k, :], rhs=fusedT[:, k, :],
                                 start=(k == 0), stop=(k == NC3 - 1))
            nc.scalar.activation(hT_t[:, m, :], h_ps[:], AF.Relu,
                                 bias=bf1_t[:, m : m + 1])

        yT_t = tail_p.tile([128, ND, BC], f32)
        sq_t = tail_p.tile([128, ND, BC], f32)
        for m in range(ND):
            wt = wf1_p.tile([128, ND, 128], bf16, tag="wf2")
            nc.gpsimd.dma_start(wt[:], wf2_d[m].rearrange("p (k c) -> p k c", c=128))
            y_ps = ps_s.tile([128, BC], f32, tag="pss")
            for k in range(ND):
                nc.tensor.matmul(y_ps[:], lhsT=wt[:, k, :], rhs=hT_t[:, k, :],
                                 start=(k == 0), stop=(k == ND - 1))
            nc.vector.tensor_scalar_add(yT_t[:, m, :], y_ps[:], bf2_t[:, m : m + 1])
            nc.scalar.square(sq_t[:, m, :], yT_t[:, m, :])

        sum_ps = ps_s.tile([1, BC], f32, tag="pss")
        for m in range(ND):
            nc.tensor.matmul(sum_ps[:], lhsT=ones_col[:], rhs=yT_t[:, m, :],
                             start=(m == 0), stop=(m == ND - 1))
        ssq_ps = ps_s.tile([1, BC], f32, tag="pss")
        for m in range(ND):
            nc.tensor.matmul(ssq_ps[:], lhsT=ones_col[:], rhs=sq_t[:, m, :],
                             start=(m == 0), stop=(m == ND - 1))
        mean_t = sm_p.tile([1, BC], f32, tag="mean")
        nc.scalar.mul(mean_t[:], sum_ps[:], 1.0 / D)
        msq_t = sm_p.tile([1, BC], f32, tag="msq")
        nc.scalar.mul(msq_t[:], ssq_ps[:], 1.0 / D)
        m2_t = sm_p.tile([1, BC], f32, tag="m2")
        nc.vector.tensor_mul(m2_t[:], mean_t[:], mean_t[:])
        var_t = sm_p.tile([1, BC], f32, tag="var")
        nc.vector.tensor_sub(var_t[:], msq_t[:], m2_t[:])
        nc.vector.tensor_scalar_add(var_t[:], var_t[:], LN_EPS)
        sd_t = sm_p.tile([1, BC], f32, tag="sd")
        nc.scalar.sqrt(sd_t[:], var_t[:])
        rstd_t = sm_p.tile([1, BC], f32, tag="rstd")
        nc.vector.reciprocal(rstd_t[:], sd_t[:])

        mb_ps = ps_s.tile([128, BC], f32, tag="pss")
        nc.tensor.matmul(mb_ps[:], lhsT=ones_f[:], rhs=mean_t[:],
                         start=True, stop=True)
        mb_t = sm_p.tile([128, BC], f32, tag="mbt")
        nc.vector.tensor_copy(mb_t[:], mb_ps[:])
        rb_ps = ps_s.tile([128, BC], f32, tag="pss")
        nc.tensor.matmul(rb_ps[:], lhsT=ones_f[:], rhs=rstd_t[:],
                         start=True, stop=True)
        rb_t = sm_p.tile([128, BC], f32, tag="rbt")
        nc.vector.tensor_copy(rb_t[:], rb_ps[:])

        zrow_t = tail_p.tile([BC, D], f32)
        for m in range(ND):
            z_t = tmp_p.tile([128, BC], f32, tag="z")
            nc.vector.tensor_sub(z_t[:], yT_t[:, m, :], mb_t[:])
            nc.vector.tensor_mul(z_t[:], z_t[:], rb_t[:])
            z2_t = tmp_p.tile([128, BC], f32, tag="z2")
            nc.vector.tensor_scalar(
                z2_t[:], z_t[:], scalar1=lng_t[:, m : m + 1],
                scalar2=lnb_t[:, m : m + 1], op0=ALU.mult, op1=ALU.add,
            )
            tr_ps = ps_s.tile([BC, 128], f32, tag="pss")
            nc.tensor.transpose(tr_ps[:], z2_t[:], ident_t[:])
            nc.vector.tensor_copy(zrow_t[:, m * 128 : (m + 1) * 128], tr_ps[:])
        nc.sync.dma_start(out_d[:, :], zrow_t[:, :])

    nc.compile()
    return nc


def _host_masks(inputs):
    x_ids = np.asarray(inputs["x_ids"])
    pad_idx = int(np.asarray(inputs["pad_idx"]))
    sep_idx = int(np.asarray(inputs["sep_idx"]))
    valid = x_ids != pad_idx
    sepm = x_ids == sep_idx
    has = sepm.any(axis=1)
    first = sepm.argmax(axis=1)
    vlen = valid.sum(axis=1)
    fb = np.clip(vlen // 2, 1, max(1, L - 2))
    sp = np.where(has, first, fb)
    pos = np.arange(L)
    fmask = ((pos[None, :] < sp[:, None]) & valid).astype(np.float32)
    omask = (pos[None, :] > sp[:, None]) & valid
    obias = np.where(omask, 0.0, OBIAS_RAW).astype(np.float32)

    F_all = np.ceil(sp / 128).astype(int)
    J0_all = np.minimum((sp + 1) // 128, NL)
    bounds = tuple(
        (int(F_all.reshape(NCORES, BC)[:, s].max()),
         int(J0_all.reshape(NCORES, BC)[:, s].min()))
        for s in range(BC)
    )
    return fmask, obias, bounds


def _host_prep_generic(inputs, fmask, obias):
    x = np.asarray(inputs["x"], dtype=np.float32)
    assert x.shape == (B, L, D), x.shape
    xT = np.ascontiguousarray(x.transpose(0, 2, 1))

    def w(name):
        return np.ascontiguousarray(np.asarray(inputs[name], dtype=np.float32))

    def ppart(name):
        return np.ascontiguousarray(np.asarray(inputs[name], dtype=np.float32)
                                    .reshape(ND, 128).T)

    use_m = all(not np.any(np.asarray(inputs[n])) for n in PBIAS_NAMES)
    shared = {}
    if use_m:
        for dst, qn, kn in (("m_sup", "w_sq", "w_sk"), ("m_con", "w_cq", "w_ck"),
                            ("m_rep", "w_rq", "w_rk")):
            shared[dst] = _m_matrix(inputs[qn], inputs[kn]).astype(np_bf16)
    else:
        for p in range(6):
            shared[PROJ_NAMES[p]] = w(PROJ_NAMES[p]).astype(np_bf16)
            shared[PBIAS_NAMES[p]] = w(PBIAS_NAMES[p]).reshape(1, D).astype(np_bf16)
    shared["w_anom"] = w("w_anom").reshape(D, 1).astype(np_bf16)

    def mpack(name, nk):
        a = w(name)                      # [nk*128, ND*128]
        a = a.reshape(nk, 128, ND, 128).transpose(2, 1, 0, 3).reshape(ND, 128, nk * 128)
        return np.ascontiguousarray(a).astype(np_bf16)

    shared["w_f1"] = mpack("w_f1", NC3)
    shared["w_f2"] = mpack("w_f2", ND)
    shared["b_f1"] = ppart("b_f1")
    shared["b_f2"] = ppart("b_f2")
    shared["ln_g"] = ppart("ln_g")
    shared["ln_b"] = ppart("ln_b")

    in_maps = []
    for c in range(NCORES):
        sl = slice(c * BC, (c + 1) * BC)
        m = dict(shared)
        m["x"] = np.ascontiguousarray(x[sl])
        m["xT"] = np.ascontiguousarray(xT[sl]).astype(np_bf16)
        m["fmask"] = np.ascontiguousarray(fmask[sl])
        m["obias"] = np.ascontiguousarray(obias[sl]).astype(np_bf16)
        in_maps.append(m)
    return in_maps, use_m


import os

USE_FP8 = bool(int(os.environ.get("KFP8", "0")))


def run(trace=False, **inputs):
    fmask, obias, bounds = _host_masks(inputs)
    use_m = all(not np.any(np.asarray(inputs[n])) for n in PBIAS_NAMES)
    if use_m and fast_eligible(bounds):
        key = ("fast", bounds, USE_FP8)
        if key not in _PROGRAM_CACHE:
            _PROGRAM_CACHE[key] = build_program_fast(bounds, use_fp8=USE_FP8)
        nc = _PROGRAM_CACHE[key]
        in_maps = _host_prep_fast(inputs, fmask, obias, bounds,
                                  use_fp8=USE_FP8)
    else:
        key = ("gen", bounds, use_m)
        if key not in _PROGRAM_CACHE:
            _PROGRAM_CACHE[key] = build_program(bounds, use_m=use_m)
        nc = _PROGRAM_CACHE[key]
        in_maps, use_m = _host_prep_generic(inputs, fmask, obias)
    res = bass_utils.run_bass_kernel_spmd(
        nc, in_maps, core_ids=list(range(NCORES)), trace=trace
    )
    out = np.concatenate([res.results[c]["out"] for c in range(NCORES)], axis=0)
    return out.astype(np.float32), res


def kernel(**inputs):
    out, _ = run(trace=False, **inputs)
    return out
